# revision 8
# baseline (speedup 1.0000x reference)
# Trainium2 Bass kernel for nn_BackboneDEARI (BRITS/DEARI-style imputation RNN).
#
# Problem: B=256, T=128, F=256, H=512.  Per timestep (sequential over T):
#   gamma_h = exp(-relu(d_t @ W_gh.T + b_gh));  h = h * gamma_h
#   x_h     = h @ W_hist.T + b_hist
#   x_r     = m*x + (1-m)*x_h
#   xu      = x_r @ W_fr_m.T + b_fr          (W_fr with zeroed diagonal)
#   gamma_x = exp(-relu(d*w_gx + b_gx))
#   beta    = sigmoid([gamma_x; m] @ W_comb.T + b_comb)
#   x_comb  = beta*xu + (1-beta)*x_h         (-> "reconstruction" output)
#   x_imp   = m*x + (1-m)*x_comb
#   LSTM: gates = [x_imp; m] @ W_ih.T + b_ih + h @ W_hh.T + b_hh
#
# Strategy:
#  - Pure data parallelism: B=256 sharded 8 ways (32 per core), params replicated,
#    no collectives.
#  - Layout: "features on partitions" everywhere.  Every activation is stored
#    transposed [feat_chunk(128 part), batch(free)] so every GEMM is
#    out = W_chunk.T.T @ act  with the weight as the stationary operand and no
#    transposes anywhere.
#  - gamma_h, gamma_x, beta depend only on inputs -> precomputed for all T in
#    large-N GEMMs/activation sweeps, staged to DRAM scratch, streamed back per
#    8-step block during the recurrence.
#  - Loss and x_imp are reconstructed on the host from the kernel's
#    reconstruction output (x_imp = m*x + (1-m)*rec; loss from rec, x, m).
#
# kernel(**inputs) -> (x_imp, reconstruction, h, x_loss, kl) matching reference.

import sys

sys.path.insert(0, "/opt/trn_rl_repo")

import numpy as np
import ml_dtypes

import concourse.bass as bass
import concourse.tile as tile
from concourse import mybir, bacc
from concourse import bass_utils
from concourse.bass import ds

F32 = mybir.dt.float32
BF16 = mybir.dt.bfloat16

B, T, F, H = 256, 128, 256, 512
NCORES = 8
BC = B // NCORES        # batch per core = 32
FC = F // 128           # 2 feature chunks
HC = H // 128           # 4 hidden chunks
G4 = 4 * H // 128       # 16 gate output chunks
NT = T * BC             # 4096 columns in precompute layout
BLK = 8                 # timesteps per DMA block
NTILE = 512             # free-dim tile for precompute GEMMs


def _np_dt(my):
    return {F32: np.float32, BF16: ml_dtypes.bfloat16}[my]


class Cfg:
    """dtype configuration: 'g' = recurrent GEMM stream dtype (weights +
    activations fed to the tensor engine), 'pg' = precompute GEMM dtype."""

    def __init__(self, g=BF16, pg=BF16):
        self.g = g
        self.pg = pg

    @property
    def key(self):
        return (str(self.g), str(self.pg))


def build_program(cfg: Cfg, t_steps=T):
    """Builds the Bass program; returns (nc, input_names)."""
    g = cfg.g
    pg = cfg.pg
    nt = t_steps * BC

    nc = bacc.Bacc(
        "TRN2",
        target_bir_lowering=False,
        debug=False,
        enable_asserts=False,
        num_devices=NCORES,
    )

    # ---------------- DRAM I/O ----------------
    # streamed per-core input shards, feature-major: [chunk, 128, t*BC+b]
    d_dr = nc.dram_tensor("d_in", [FC, 128, nt], pg, kind="ExternalInput").ap()
    x_dr = nc.dram_tensor("x_in", [FC, 128, nt], g, kind="ExternalInput").ap()
    m_dr = nc.dram_tensor("m_in", [FC, 128, nt], g, kind="ExternalInput").ap()
    m8_dr = nc.dram_tensor("m8_in", [FC, 128, nt], mybir.dt.uint8,
                           kind="ExternalInput").ap()

    # weights, pre-transposed on host: for W [O, I] we pass W.T reshaped
    # [I/128, 128, O]
    wgh_dr = nc.dram_tensor("wgh", [FC, 128, H], pg, kind="ExternalInput").ap()
    wcomb_dr = nc.dram_tensor("wcomb", [2 * FC, 128, F], pg, kind="ExternalInput").ap()
    whist_dr = nc.dram_tensor("whist", [HC, 128, F], g, kind="ExternalInput").ap()
    wfr_dr = nc.dram_tensor("wfr", [FC, 128, F], g, kind="ExternalInput").ap()
    wih_dr = nc.dram_tensor("wih", [2 * FC, 128, 4 * H], g, kind="ExternalInput").ap()
    whh_dr = nc.dram_tensor("whh", [HC, 128, 4 * H], g, kind="ExternalInput").ap()

    # biases (fp32), packed per-partition: [128, nchunks]
    bgh_dr = nc.dram_tensor("bgh", [128, HC], F32, kind="ExternalInput").ap()
    bgx_dr = nc.dram_tensor("bgx", [128, FC], F32, kind="ExternalInput").ap()
    wgx_dr = nc.dram_tensor("wgx", [128, FC], F32, kind="ExternalInput").ap()
    bcomb_dr = nc.dram_tensor("bcomb", [128, FC], F32, kind="ExternalInput").ap()
    bhist_dr = nc.dram_tensor("bhist", [128, FC], F32, kind="ExternalInput").ap()
    bfr_dr = nc.dram_tensor("bfr", [128, FC], F32, kind="ExternalInput").ap()
    # gate bias (b_ih + b_hh) broadcast over batch: [128, 4 gates, HC, BC]
    bih_dr = nc.dram_tensor("bih", [128, 4, HC, BC], F32, kind="ExternalInput").ap()

    # outputs
    rec_dr = nc.dram_tensor("rec", [FC, 128, nt], F32, kind="ExternalOutput").ap()
    hout_dr = nc.dram_tensor("hout", [128, HC, BC], F32, kind="ExternalOutput").ap()

    # scratch (device-local)
    gh_dr = nc.dram_tensor("gh_s", [HC, 128, nt], g, kind="Internal").ap()
    beta_dr = nc.dram_tensor("beta_s", [FC, 128, nt], F32, kind="Internal").ap()

    input_names = [
        "d_in", "x_in", "m_in", "m8_in", "wgh", "wcomb", "whist", "wfr", "wih", "whh",
        "bgh", "bgx", "wgx", "bcomb", "bhist", "bfr", "bih",
    ]

    with tile.TileContext(nc) as tc:
        from contextlib import ExitStack
        with ExitStack() as _es:
            _body(tc, _es, locals(), cfg, t_steps)

    nc.compile()
    return nc, input_names


def _body(tc, es, v, cfg, t_steps):
    nc = tc.nc
    g = cfg.g
    pg = cfg.pg
    AF = mybir.ActivationFunctionType
    sync = nc.sync

    d_dr, x_dr, m_dr, m8_dr = v["d_dr"], v["x_dr"], v["m_dr"], v["m8_dr"]
    gh_dr, beta_dr, rec_dr, hout_dr = v["gh_dr"], v["beta_dr"], v["rec_dr"], v["hout_dr"]

    # ------------- persistent weights / biases in SBUF -------------
    wpool = es.enter_context(tc.tile_pool(name="weights", bufs=1))
    wgh_sb = wpool.tile([128, FC, H], pg)
    wcomb_sb = wpool.tile([128, 2 * FC, F], pg)
    whist_sb = wpool.tile([128, HC, F], g)
    wfr_sb = wpool.tile([128, FC, F], g)
    wih_sb = wpool.tile([128, 2 * FC, 4 * H], g)
    whh_sb = wpool.tile([128, HC, 4 * H], g)
    bgh_sb = wpool.tile([128, HC], F32)
    bgx_sb = wpool.tile([128, FC], F32)
    wgx_sb = wpool.tile([128, FC], F32)
    bcomb_sb = wpool.tile([128, FC], F32)
    bhist_sb = wpool.tile([128, FC], F32)
    bfr_sb = wpool.tile([128, FC], F32)
    bih_sb = wpool.tile([128, 4, HC, BC], F32)

    for sb, dr in [
        (wgh_sb, v["wgh_dr"]), (wcomb_sb, v["wcomb_dr"]), (whist_sb, v["whist_dr"]),
        (wfr_sb, v["wfr_dr"]), (wih_sb, v["wih_dr"]), (whh_sb, v["whh_dr"]),
    ]:
        sync.dma_start(out=sb, in_=dr.rearrange("c p o -> p c o"))
    for sb, dr in [
        (bgh_sb, v["bgh_dr"]), (bgx_sb, v["bgx_dr"]), (wgx_sb, v["wgx_dr"]),
        (bcomb_sb, v["bcomb_dr"]), (bhist_sb, v["bhist_dr"]), (bfr_sb, v["bfr_dr"]),
        (bih_sb, v["bih_dr"]),
    ]:
        sync.dma_start(out=sb, in_=dr)

    # ------------- phase 1: precompute gamma_h, beta for all t -------------
    nt = t_steps * BC
    n_ntiles = (nt + NTILE - 1) // NTILE
    with tc.tile_pool(name="pre", bufs=2) as pre, \
         tc.tile_pool(name="pre_ps", bufs=2, space="PSUM") as pre_ps:
        for it in range(n_ntiles):
            w = min(NTILE, nt - it * NTILE)
            cols = ds(it * NTILE, w)
            dtl = []
            mtl = []
            for fc in range(FC):
                dt_ = pre.tile([128, NTILE], pg, tag=f"d{fc}")
                sync.dma_start(out=dt_[:, :w], in_=d_dr[fc, :, cols])
                dtl.append(dt_)
                mt_ = pre.tile([128, NTILE], g, tag=f"m{fc}")
                sync.dma_start(out=mt_[:, :w], in_=m_dr[fc, :, cols])
                mtl.append(mt_)
            # gamma_h = exp(-relu(W_gh @ d + b_gh))
            for hc in range(HC):
                ps = pre_ps.tile([128, NTILE], F32, tag="ps_gh")
                for fc in range(FC):
                    nc.tensor.matmul(
                        ps[:, :w],
                        wgh_sb[:, fc, hc * 128:(hc + 1) * 128],
                        dtl[fc][:, :w],
                        start=(fc == 0), stop=(fc == FC - 1),
                    )
                tmp = pre.tile([128, NTILE], F32, tag="tmp_gh")
                nc.scalar.activation(tmp[:, :w], ps[:, :w], AF.Relu,
                                     bias=bgh_sb[:, hc:hc + 1])
                ght = pre.tile([128, NTILE], g, tag="ght")
                nc.scalar.activation(ght[:, :w], tmp[:, :w], AF.Exp, scale=-1.0)
                sync.dma_start(out=gh_dr[hc, :, cols], in_=ght[:, :w])
            # gamma_x = exp(-relu(d*w_gx + b_gx))  (kept in SBUF for beta)
            gxl = []
            for fc in range(FC):
                t1 = pre.tile([128, NTILE], F32, tag=f"gx_t{fc}")
                nc.scalar.activation(t1[:, :w], dtl[fc][:, :w], AF.Relu,
                                     scale=wgx_sb[:, fc:fc + 1],
                                     bias=bgx_sb[:, fc:fc + 1])
                gxt = pre.tile([128, NTILE], pg, tag=f"gx{fc}")
                nc.scalar.activation(gxt[:, :w], t1[:, :w], AF.Exp, scale=-1.0)
                gxl.append(gxt)
            # beta = sigmoid(W_comb @ [gamma_x; m] + b_comb)
            for fc in range(FC):
                ps = pre_ps.tile([128, NTILE], F32, tag="ps_b")
                for k in range(FC):
                    nc.tensor.matmul(
                        ps[:, :w], wcomb_sb[:, k, fc * 128:(fc + 1) * 128],
                        gxl[k][:, :w], start=(k == 0), stop=False)
                for k in range(FC):
                    nc.tensor.matmul(
                        ps[:, :w], wcomb_sb[:, FC + k, fc * 128:(fc + 1) * 128],
                        mtl[k][:, :w], start=False, stop=(k == FC - 1))
                bt = pre.tile([128, NTILE], F32, tag="bt")
                nc.scalar.activation(bt[:, :w], ps[:, :w], AF.Sigmoid,
                                     bias=bcomb_sb[:, fc:fc + 1])
                sync.dma_start(out=beta_dr[fc, :, cols], in_=bt[:, :w])

    # ------------- phase 2: recurrence -------------
    state = es.enter_context(tc.tile_pool(name="state", bufs=1))
    h_sb = state.tile([128, HC, BC], F32)
    c_sb = state.tile([128, HC, BC], F32)
    nc.vector.memset(h_sb, 0.0)
    nc.vector.memset(c_sb, 0.0)

    stream = es.enter_context(tc.tile_pool(name="stream", bufs=2))
    work = es.enter_context(tc.tile_pool(name="work", bufs=2))
    psp = es.enter_context(tc.tile_pool(name="ps", bufs=1, space="PSUM"))
    psg = es.enter_context(tc.tile_pool(name="psg", bufs=1, space="PSUM"))

    n_blk = t_steps // BLK
    # gate order: process i(0), g(2), f(1), o(3); o last frees the tail
    gate_order = [0, 2, 1, 3]
    gate_fn = {0: AF.Sigmoid, 1: AF.Sigmoid, 2: AF.Tanh, 3: AF.Sigmoid}

    for blk in range(n_blk):
        cols = ds(blk * BLK * BC, BLK * BC)
        gh_blk = stream.tile([128, BLK, HC, BC], g, tag="gh_blk")
        for c in range(HC):
            sync.dma_start(out=gh_blk[:, :, c, :],
                           in_=gh_dr[c, :, cols].rearrange("p (s b) -> p s b", b=BC))
        beta_blk = stream.tile([128, BLK, FC, BC], F32, tag="beta_blk")
        x_blk = stream.tile([128, BLK, FC, BC], g, tag="x_blk")
        m_blk = stream.tile([128, BLK, FC, BC], g, tag="m_blk")
        m8_blk = stream.tile([128, BLK, FC, BC], mybir.dt.uint8, tag="m8_blk")
        for c in range(FC):
            sync.dma_start(out=beta_blk[:, :, c, :],
                           in_=beta_dr[c, :, cols].rearrange("p (s b) -> p s b", b=BC))
            sync.dma_start(out=x_blk[:, :, c, :],
                           in_=x_dr[c, :, cols].rearrange("p (s b) -> p s b", b=BC))
            sync.dma_start(out=m_blk[:, :, c, :],
                           in_=m_dr[c, :, cols].rearrange("p (s b) -> p s b", b=BC))
            sync.dma_start(out=m8_blk[:, :, c, :],
                           in_=m8_dr[c, :, cols].rearrange("p (s b) -> p s b", b=BC))
        rec_blk = stream.tile([128, BLK, FC, BC], F32, tag="rec_blk")

        for j in range(BLK):
            ghj = gh_blk[:, j]     # [128, HC, BC] contiguous
            bj = beta_blk[:, j]    # [128, FC, BC]
            xj = x_blk[:, j]
            mj = m_blk[:, j]
            mj8 = m8_blk[:, j]

            # h_dec = h * gamma_h   (cast to GEMM dtype on write)
            h_dec = work.tile([128, HC, BC], g, tag="h_dec")
            nc.vector.tensor_mul(h_dec, h_sb, ghj)

            # x_h = W_hist @ h_dec + b_hist
            ps_xh = psp.tile([128, FC, BC], F32, tag="ps_xh")
            for fc in range(FC):
                for hc in range(HC):
                    nc.tensor.matmul(
                        ps_xh[:, fc, :], whist_sb[:, hc, fc * 128:(fc + 1) * 128],
                        h_dec[:, hc, :], start=(hc == 0), stop=(hc == HC - 1))
            x_h = work.tile([128, FC, BC], F32, tag="x_h")
            for fc in range(FC):
                nc.scalar.activation(x_h[:, fc, :], ps_xh[:, fc, :], AF.Identity,
                                     bias=bhist_sb[:, fc:fc + 1])

            # x_r = m ? x : x_h
            x_r = work.tile([128, FC, BC], g, tag="x_r")
            nc.scalar.copy(x_r, x_h)
            nc.vector.copy_predicated(x_r[:, 0:FC, 0:BC], mj8, xj)

            # gates, early part: W_ih[m-part] @ m + W_hh @ h_dec.
            # Each (gate, out-chunk) accumulation group is contiguous and is
            # CLOSED here; the x_imp contribution accumulates in separate
            # "late" banks and is summed in during the bias add (PSUM
            # accumulation groups cannot interleave within a bank).
            ps_gates = {}
            def emit_gate_early(gi):
                ps = psg.tile([128, HC, BC], F32, tag=f"ps_g{gi}")
                ps_gates[gi] = ps
                for oc2 in range(HC):
                    oc = gi * HC + oc2
                    for k in range(FC):
                        nc.tensor.matmul(
                            ps[:, oc2, :], wih_sb[:, FC + k, oc * 128:(oc + 1) * 128],
                            mj[:, k], start=(k == 0), stop=False)
                    for hc in range(HC):
                        nc.tensor.matmul(
                            ps[:, oc2, :], whh_sb[:, hc, oc * 128:(oc + 1) * 128],
                            h_dec[:, hc, :], start=False, stop=(hc == HC - 1))

            emit_gate_early(0)

            # xu = W_fr_m @ x_r + b_fr   (emitted after i-gate early work so the
            # PE has work while the x_r chain completes)
            ps_xu = psp.tile([128, FC, BC], F32, tag="ps_xu")
            for fc in range(FC):
                for k in range(FC):
                    nc.tensor.matmul(
                        ps_xu[:, fc, :], wfr_sb[:, k, fc * 128:(fc + 1) * 128],
                        x_r[:, k, :], start=(k == 0), stop=(k == FC - 1))

            for gi in gate_order[1:]:
                emit_gate_early(gi)

            # x_comb = x_h + beta*(xu + b_fr - x_h)  -> rec output
            xu_b = work.tile([128, FC, BC], F32, tag="xu_b")
            for fc in range(FC):
                nc.scalar.activation(xu_b[:, fc, :], ps_xu[:, fc, :], AF.Identity,
                                     bias=bfr_sb[:, fc:fc + 1])
            dsub = work.tile([128, FC, BC], F32, tag="dsub")
            nc.vector.tensor_sub(dsub, xu_b, x_h)
            dmul = work.tile([128, FC, BC], F32, tag="dmul")
            nc.vector.tensor_mul(dmul, bj, dsub)
            rec_j = rec_blk[:, j]
            nc.vector.tensor_add(rec_j, x_h, dmul)

            # x_imp = m ? x : x_comb
            x_imp = work.tile([128, FC, BC], g, tag="x_imp")
            nc.scalar.copy(x_imp, rec_j)
            nc.vector.copy_predicated(x_imp[:, 0:FC, 0:BC], mj8, xj)

            # gates, late part: W_ih[x-part] @ x_imp into two separate banks
            # (first bank: gates 0,2; second: gates 1,3) so early gates'
            # combining can start while the PE finishes the second bank.
            ps_late = {}
            for half, gis in enumerate((gate_order[:2], gate_order[2:])):
                psl = psg.tile([128, 2 * HC, BC], F32, tag=f"ps_late{half}")
                for li, gi in enumerate(gis):
                    ps_late[gi] = psl[:, li * HC:(li + 1) * HC, :]
                    for oc2 in range(HC):
                        oc = gi * HC + oc2
                        for k in range(FC):
                            nc.tensor.matmul(
                                psl[:, li * HC + oc2, :],
                                wih_sb[:, k, oc * 128:(oc + 1) * 128],
                                x_imp[:, k, :], start=(k == 0), stop=(k == FC - 1))

            # bias + late sum + activation per gate
            gate_sb = {}
            for gi in gate_order:
                pre_a = work.tile([128, HC, BC], F32, tag=f"pre{gi}")
                nc.vector.tensor_add(pre_a, ps_gates[gi], bih_sb[:, gi])
                nc.vector.tensor_add(pre_a, pre_a, ps_late[gi])
                gs = work.tile([128, HC, BC], F32, tag=f"gact{gi}")
                nc.scalar.activation(gs, pre_a, gate_fn[gi])
                gate_sb[gi] = gs

            # c = f*c + i*g ; h = o*tanh(c)
            ig = work.tile([128, HC, BC], F32, tag="ig")
            nc.vector.tensor_mul(ig, gate_sb[0], gate_sb[2])
            fc_ = work.tile([128, HC, BC], F32, tag="fc_")
            nc.vector.tensor_mul(fc_, gate_sb[1], c_sb)
            nc.vector.tensor_add(c_sb, fc_, ig)
            th = work.tile([128, HC, BC], F32, tag="th")
            nc.scalar.activation(th, c_sb, AF.Tanh)
            nc.vector.tensor_mul(h_sb, gate_sb[3], th)

        for c in range(FC):
            sync.dma_start(
                out=rec_dr[c, :, cols].rearrange("p (s b) -> p s b", b=BC),
                in_=rec_blk[:, :, c, :])

    sync.dma_start(out=hout_dr, in_=h_sb)


# ------------------------------------------------------------------
# host side
# ------------------------------------------------------------------

def _prep_inputs(inputs, cfg: Cfg, t_steps=T):
    """Returns list of per-core in_maps."""
    gnp = _np_dt(cfg.g)
    pgnp = _np_dt(cfg.pg)
    x = np.asarray(inputs["x"], np.float32)
    m = np.asarray(inputs["mask"], np.float32)
    d = np.asarray(inputs["deltas"], np.float32)
    if t_steps != T:
        x, m, d = x[:, :t_steps], m[:, :t_steps], d[:, :t_steps]

    def shard_feat_major(a, dt):
        # [B, t, F] -> per core [FC, 128, t*BC]  with col = t*BC + b
        a = a.reshape(NCORES, BC, t_steps, F).transpose(0, 3, 2, 1)  # [c, F, t, BC]
        a = np.ascontiguousarray(a).reshape(NCORES, FC, 128, t_steps * BC)
        return a.astype(dt)

    xs = shard_feat_major(x, gnp)
    ms = shard_feat_major(m, gnp)
    m8s = shard_feat_major((m > 0).astype(np.float32), np.uint8)
    dsh = shard_feat_major(d, pgnp)

    def wt(wname, dt):
        w = np.asarray(inputs[wname], np.float32)
        if wname == "W_fr":
            w = w * (1.0 - np.eye(F, dtype=np.float32))
        wT = np.ascontiguousarray(w.T)  # [I, O]
        I, O = wT.shape
        return wT.reshape(I // 128, 128, O).astype(dt)

    def bias_pp(bname):
        b = np.asarray(inputs[bname], np.float32)
        return np.ascontiguousarray(b.reshape(-1, 128).T)  # [128, nchunks]

    wgh = wt("W_gh", pgnp)
    wcomb = wt("W_comb", pgnp)
    whist = wt("W_hist", gnp)
    wfr = wt("W_fr", gnp)
    wih = wt("W_ih", gnp)
    whh = wt("W_hh", gnp)

    bih_t = (np.asarray(inputs["b_ih"], np.float32)
             + np.asarray(inputs["b_hh"], np.float32))
    # [4H] -> [128, 4 gates, HC, BC] broadcast over batch
    bih = bih_t.reshape(4, HC, 128).transpose(2, 0, 1)  # [128, 4, HC]
    bih = np.ascontiguousarray(
        np.broadcast_to(bih[:, :, :, None], (128, 4, HC, BC))).astype(np.float32)

    shared = dict(
        wgh=wgh, wcomb=wcomb, whist=whist, wfr=wfr, wih=wih, whh=whh,
        bgh=bias_pp("b_gh"), bgx=bias_pp("b_gx"), wgx=bias_pp("w_gx"),
        bcomb=bias_pp("b_comb"), bhist=bias_pp("b_hist"), bfr=bias_pp("b_fr"),
        bih=bih,
    )
    in_maps = []
    for c in range(NCORES):
        im = dict(shared)
        im["d_in"] = dsh[c]
        im["x_in"] = xs[c]
        im["m_in"] = ms[c]
        im["m8_in"] = m8s[c]
        in_maps.append(im)
    return in_maps


def _assemble(results, inputs, t_steps=T):
    x = np.asarray(inputs["x"], np.float32)[:, :t_steps]
    m = np.asarray(inputs["mask"], np.float32)[:, :t_steps]
    recs = []
    hs = []
    for c in range(NCORES):
        r = results[c]["rec"].reshape(FC, 128, t_steps, BC)
        recs.append(np.ascontiguousarray(r.transpose(3, 2, 0, 1)).reshape(BC, t_steps, F))
        ho = results[c]["hout"]  # [128, HC, BC]
        hs.append(np.ascontiguousarray(ho.transpose(2, 1, 0)).reshape(BC, H))
    rec = np.concatenate(recs, axis=0)
    h = np.concatenate(hs, axis=0)
    x_imp = m * x + (1.0 - m) * rec
    num = np.abs(rec - x) * m
    num_t = num.sum(axis=(0, 2), dtype=np.float32)
    den_t = m.sum(axis=(0, 2), dtype=np.float32)
    x_loss = np.float32((num_t / (den_t + 1e-12)).sum(dtype=np.float32))
    kl = np.zeros((), np.float32)
    return (x_imp.astype(np.float32), rec.astype(np.float32), h.astype(np.float32),
            x_loss, kl)


_CACHE = {}


def _get_program(cfg: Cfg, t_steps=T):
    key = (cfg.key, t_steps)
    if key not in _CACHE:
        _CACHE[key] = build_program(cfg, t_steps)
    return _CACHE[key]


def run(inputs, cfg: Cfg, t_steps=T, trace=False):
    nc, _ = _get_program(cfg, t_steps)
    in_maps = _prep_inputs(inputs, cfg, t_steps)
    res = bass_utils.run_bass_kernel_spmd(
        nc, in_maps, core_ids=list(range(NCORES)), trace=trace)
    return _assemble(res.results, inputs, t_steps), res


def kernel(**inputs):
    (out, _res) = run(inputs, Cfg())
    return out


if __name__ == "__main__":
    # smoke build only
    cfg = Cfg()
    nc, names = build_program(cfg, t_steps=8)
    print("built ok", len(nc.m.functions[0].instructions)
          if hasattr(nc.m.functions[0], "instructions") else "")
